# revision 1
# baseline (speedup 1.0000x reference)
"""Trainium2 Bass kernel for an 8-expert MoE FFN layer (nn_MoELayer).

Reference computation (per expert e over its contiguous 1024-token chunk):
    h = gelu(x_e @ w1[e] + b1[e]);  y_e = h @ w2[e] + b2[e]

Sharding: expert parallelism — core e holds expert e's weights and its token
chunk (the gate yields equal contiguous chunks, so no all-to-all is needed).
Each core runs the same SPMD program on its own data.

Per-core kernel (T=1024 tokens, D=1024, F=4096), all matmuls in float32r
(TF32-like, 1 cycle/row on the PE vs 4 for fp32):
  phase 1: for each 128-wide f-tile: h^T[ft] = gelu(w1[:,ft]^T @ x^T + b1[ft])
           (f on partitions -> b1 is a per-partition ACT bias; h^T resident in SBUF)
  phase 2: for each 128-wide dm-tile: y^T[dmo] = w2[:,dmo]^T @ h^T + b2[dmo]
           (dm-tile outer -> one 2-bank PSUM accumulator at a time)
All layout transposes/repacks are done on the host so every DMA is a large
partition-contiguous stream.
"""

import numpy as np

E = 8          # experts == cores
B, S = 2, 4096
D = 1024       # d_model
F = 4096       # d_ff
T = (B * S) // E  # tokens per expert chunk = 1024
P = 128
DO = D // P    # 8  k-tiles of d_model
FT = F // P    # 32 f-tiles of d_ff
DMO = D // P   # 8  output dm-tiles
FT2 = FT // 2  # half-slab of w2 f-tiles
NCHUNK = T // 512  # 2 moving-operand chunks (fp32 moving max is 512)

_cached = None


def _build():
    import concourse.mybir as mybir
    import concourse.tile as tile
    from concourse import bacc

    f32 = mybir.dt.float32
    f32r = mybir.dt.float32r

    nc = bacc.Bacc("TRN2", target_bir_lowering=False, debug=False, num_devices=E)

    xT_d = nc.dram_tensor("xT", [P, DO, T], f32r, kind="ExternalInput")
    w1_d = nc.dram_tensor("w1r", [FT, P, DO, P], f32r, kind="ExternalInput")
    b1_d = nc.dram_tensor("b1r", [P, FT], f32, kind="ExternalInput")
    w2_d = nc.dram_tensor("w2r", [DMO, 2, P, FT2, P], f32r, kind="ExternalInput")
    b2_d = nc.dram_tensor("b2r", [P, DMO], f32, kind="ExternalInput")
    yT_d = nc.dram_tensor("yT", [DMO, P, T], f32, kind="ExternalOutput")

    gelu = mybir.ActivationFunctionType.Gelu_apprx_tanh

    with tile.TileContext(nc) as tc:
        with (
            tc.tile_pool(name="xpool", bufs=1) as xpool,
            tc.tile_pool(name="hpool", bufs=1) as hpool,
            tc.tile_pool(name="wpool", bufs=2) as wpool,
            tc.tile_pool(name="cpool", bufs=1) as cpool,
            tc.tile_pool(name="ypool", bufs=2) as ypool,
            tc.tile_pool(name="psum_h", bufs=2, space="PSUM") as psum_h,
            tc.tile_pool(name="psum_y", bufs=2, space="PSUM") as psum_y,
        ):
            b1_sb = cpool.tile([P, FT], f32)
            nc.sync.dma_start(b1_sb[:], b1_d.ap())
            b2_sb = cpool.tile([P, DMO], f32)
            nc.sync.dma_start(b2_sb[:], b2_d.ap())
            xT_sb = xpool.tile([P, DO, T], f32r)
            nc.sync.dma_start(xT_sb[:], xT_d.ap())
            h_sb = hpool.tile([P, FT, T], f32r)

            # ---- phase 1: h^T = gelu(w1^T x^T + b1), one 128-row f-tile at a time
            for ft in range(FT):
                w1_sb = wpool.tile([P, DO, P], f32r, tag="w1", name="w1_sb")
                nc.sync.dma_start(w1_sb[:], w1_d.ap()[ft])
                ph = psum_h.tile([P, T], f32, tag="ph", name="ph")
                for c in range(NCHUNK):
                    cs = slice(c * 512, (c + 1) * 512)
                    for do in range(DO):
                        nc.tensor.matmul(
                            ph[:, cs],
                            w1_sb[:, do, :],
                            xT_sb[:, do, cs],
                            start=(do == 0),
                            stop=(do == DO - 1),
                        )
                nc.scalar.activation(
                    h_sb[:, ft, :], ph[:], gelu, bias=b1_sb[:, ft : ft + 1]
                )

            # ---- phase 2: y^T[dmo] = w2[:,dmo]^T h^T + b2[dmo]
            for dmo in range(DMO):
                w2_half = []
                for hh in range(2):
                    w2_sb = wpool.tile([P, FT2, P], f32r, tag="w2", name="w2_sb")
                    nc.sync.dma_start(w2_sb[:], w2_d.ap()[dmo, hh])
                    w2_half.append(w2_sb)
                py = psum_y.tile([P, T], f32, tag="py", name="py")
                for fo in range(FT):
                    wt = w2_half[fo // FT2][:, fo % FT2, :]
                    for c in range(NCHUNK):
                        cs = slice(c * 512, (c + 1) * 512)
                        nc.tensor.matmul(
                            py[:, cs],
                            wt,
                            h_sb[:, fo, cs],
                            start=(fo == 0),
                            stop=(fo == FT - 1),
                        )
                y_sb = ypool.tile([P, T], f32, tag="y", name="y_sb")
                nc.vector.tensor_scalar_add(y_sb[:], py[:], b2_sb[:, dmo : dmo + 1])
                nc.sync.dma_start(yT_d.ap()[dmo], y_sb[:])

    nc.compile()
    return nc


def _get_nc():
    global _cached
    if _cached is None:
        _cached = _build()
    return _cached


def kernel(x, w1, b1, w2, b2):
    from concourse.bass_utils import run_bass_kernel_spmd

    nc = _get_nc()

    x = np.asarray(x, dtype=np.float32)
    w1 = np.asarray(w1, dtype=np.float32)
    b1 = np.asarray(b1, dtype=np.float32)
    w2 = np.asarray(w2, dtype=np.float32)
    b2 = np.asarray(b2, dtype=np.float32)

    tokens = x.reshape(E, T, D)
    in_maps = []
    for e in range(E):
        xT = np.ascontiguousarray(
            tokens[e].T.reshape(DO, P, T).transpose(1, 0, 2)
        )  # [p, do, t]
        w1r = np.ascontiguousarray(
            w1[e].reshape(DO, P, FT, P).transpose(2, 1, 0, 3)
        )  # [ft, p, do, j]
        b1r = np.ascontiguousarray(b1[e].reshape(FT, P).T)  # [p, ft]
        w2r = np.ascontiguousarray(
            w2[e].reshape(2, FT2, P, DMO, P).transpose(3, 0, 2, 1, 4)
        )  # [dmo, half, p, fo, j]
        b2r = np.ascontiguousarray(b2[e].reshape(DMO, P).T)  # [p, dmo]
        in_maps.append({"xT": xT, "w1r": w1r, "b1r": b1r, "w2r": w2r, "b2r": b2r})

    res = run_bass_kernel_spmd(nc, in_maps, core_ids=list(range(E)))

    out = np.empty((E, T, D), dtype=np.float32)
    for e in range(E):
        yT = res.results[e]["yT"]  # [dmo, p, t]
        out[e] = yT.transpose(2, 0, 1).reshape(T, D)
    return out.reshape(B, S, D)


# revision 3
# speedup vs baseline: 1.0314x; 1.0314x over previous
"""Trainium2 Bass kernel for an 8-expert MoE FFN layer (nn_MoELayer).

Reference computation (per expert e over its contiguous 1024-token chunk):
    h = gelu(x_e @ w1[e] + b1[e]);  y_e = h @ w2[e] + b2[e]

Sharding: expert parallelism — core e holds expert e's weights and its token
chunk (the gate yields equal contiguous chunks, so no all-to-all is needed).
Each core runs the same SPMD program on its own data.

Per-core kernel (T=1024 tokens, D=1024, F=4096), all matmuls in float32r
(TF32-like, 1 cycle/row on the PE vs 4 for fp32):
  phase 1: for each 128-wide f-tile: h^T[ft] = gelu(w1[:,ft]^T @ x^T + b1[ft])
           (f on partitions -> b1 is a per-partition ACT bias; h^T resident in SBUF)
  phase 2: for each 128-wide dm-tile: y^T[dmo] = w2[:,dmo]^T @ h^T + b2[dmo]
           (dm-tile outer -> one 2-bank PSUM accumulator at a time)
All layout transposes/repacks are done on the host so every DMA is a large
partition-contiguous stream. A short burst of dummy matmuls on scratch data
warms the PE clock (HAM) while the first input DMAs are in flight.
"""

import numpy as np

E = 8          # experts == cores
B, S = 2, 4096
D = 1024       # d_model
F = 4096       # d_ff
T = (B * S) // E  # tokens per expert chunk = 1024
P = 128
DO = D // P    # 8  k-tiles of d_model
FT = F // P    # 32 f-tiles of d_ff
DMO = D // P   # 8  output dm-tiles
FT2 = FT // 2  # half-slab of w2 f-tiles
NCHUNK = T // 512  # 2 moving-operand chunks (fp32 moving max is 512)
N_WARMUP_MM = 28

_cached = None


def _build():
    import concourse.mybir as mybir
    import concourse.tile as tile
    from concourse import bacc

    f32 = mybir.dt.float32
    f32r = mybir.dt.float32r

    nc = bacc.Bacc("TRN2", target_bir_lowering=False, debug=False, num_devices=E)

    xT_d = nc.dram_tensor("xT", [NCHUNK, P, DO, 512], f32r, kind="ExternalInput")
    w1_d = nc.dram_tensor("w1r", [FT, P, DO, P], f32r, kind="ExternalInput")
    b1_d = nc.dram_tensor("b1r", [P, FT], f32, kind="ExternalInput")
    w2_d = nc.dram_tensor("w2r", [DMO, 2, P, FT2, P], f32r, kind="ExternalInput")
    b2_d = nc.dram_tensor("b2r", [P, DMO], f32, kind="ExternalInput")
    yT_d = nc.dram_tensor("yT", [DMO, P, T], f32, kind="ExternalOutput")

    gelu = mybir.ActivationFunctionType.Gelu_apprx_tanh

    with tile.TileContext(nc) as tc:
        with (
            tc.tile_pool(name="xpool", bufs=1) as xpool,
            tc.tile_pool(name="hpool", bufs=1) as hpool,
            tc.tile_pool(name="wpool", bufs=2) as wpool,
            tc.tile_pool(name="cpool", bufs=1) as cpool,
            tc.tile_pool(name="ypool", bufs=2) as ypool,
            tc.tile_pool(name="psum_h", bufs=2, space="PSUM") as psum_h,
            tc.tile_pool(name="psum_y", bufs=2, space="PSUM") as psum_y,
        ):
            # input DMAs in critical-path order: w1[0], x chunk 0, x chunk 1, biases
            w1_tiles = {}
            w1_tiles[0] = wpool.tile([P, DO, P], f32r, tag="w1", bufs=3, name="w1_sb")
            nc.sync.dma_start(w1_tiles[0][:], w1_d.ap()[0])
            xT_sb = xpool.tile([P, DO, T], f32r)
            for c in range(NCHUNK):
                nc.sync.dma_start(
                    xT_sb[:, :, c * 512 : (c + 1) * 512], xT_d.ap()[c]
                )
            b1_sb = cpool.tile([P, FT], f32)
            nc.sync.dma_start(b1_sb[:], b1_d.ap())
            b2_sb = cpool.tile([P, DMO], f32)
            nc.sync.dma_start(b2_sb[:], b2_d.ap())

            # PE warmup: dummy matmuls on scratch while input DMAs stream.
            # Keeps the HAM clock-gate at 2.4 GHz by the time real work lands.
            scratch32 = cpool.tile([P, 512], f32)
            nc.gpsimd.memset(scratch32[:], 0.0)
            scratch = cpool.tile([P, 512], f32r)
            nc.vector.tensor_copy(scratch[:], scratch32[:])
            for i in range(N_WARMUP_MM):
                pw = psum_h.tile([P, T], f32, tag="ph", name="pwarm")
                nc.tensor.matmul(
                    pw[:, :512], scratch[:, :P], scratch[:], start=True, stop=True
                )

            h_sb = hpool.tile([P, FT, T], f32r)

            # ---- phase 1: h^T = gelu(w1^T x^T + b1), one 128-row f-tile at a time
            for ft in range(FT):
                if ft not in w1_tiles:
                    w1_tiles[ft] = wpool.tile(
                        [P, DO, P], f32r, tag="w1", bufs=3, name="w1_sb"
                    )
                    nc.sync.dma_start(w1_tiles[ft][:], w1_d.ap()[ft])
                w1_sb = w1_tiles[ft]
                ph = psum_h.tile([P, T], f32, tag="ph", name="ph")
                for c in range(NCHUNK):
                    cs = slice(c * 512, (c + 1) * 512)
                    for do in range(DO):
                        nc.tensor.matmul(
                            ph[:, cs],
                            w1_sb[:, do, :],
                            xT_sb[:, do, cs],
                            start=(do == 0),
                            stop=(do == DO - 1),
                        )
                nc.scalar.activation(
                    h_sb[:, ft, :], ph[:], gelu, bias=b1_sb[:, ft : ft + 1]
                )

            # ---- phase 2: y^T[dmo] = w2[:,dmo]^T h^T + b2[dmo]
            for dmo in range(DMO):
                w2_half = []
                for hh in range(2):
                    w2_sb = wpool.tile([P, FT2, P], f32r, tag="w2", name="w2_sb")
                    nc.sync.dma_start(w2_sb[:], w2_d.ap()[dmo, hh])
                    w2_half.append(w2_sb)
                py = psum_y.tile([P, T], f32, tag="py", name="py")
                for fo in range(FT):
                    wt = w2_half[fo // FT2][:, fo % FT2, :]
                    for c in range(NCHUNK):
                        cs = slice(c * 512, (c + 1) * 512)
                        nc.tensor.matmul(
                            py[:, cs],
                            wt,
                            h_sb[:, fo, cs],
                            start=(fo == 0),
                            stop=(fo == FT - 1),
                        )
                # bias-add + store in 512 chunks so the DMA overlaps the add
                for c in range(NCHUNK):
                    cs = slice(c * 512, (c + 1) * 512)
                    y_sb = ypool.tile([P, 512], f32, tag="y", bufs=3, name="y_sb")
                    nc.vector.tensor_scalar_add(
                        y_sb[:], py[:, cs], b2_sb[:, dmo : dmo + 1]
                    )
                    nc.sync.dma_start(yT_d.ap()[dmo, :, cs], y_sb[:])

    nc.compile()
    return nc


def _get_nc():
    global _cached
    if _cached is None:
        _cached = _build()
    return _cached


def make_in_maps(x, w1, b1, w2, b2):
    x = np.asarray(x, dtype=np.float32)
    w1 = np.asarray(w1, dtype=np.float32)
    b1 = np.asarray(b1, dtype=np.float32)
    w2 = np.asarray(w2, dtype=np.float32)
    b2 = np.asarray(b2, dtype=np.float32)

    tokens = x.reshape(E, T, D)
    in_maps = []
    for e in range(E):
        xT = np.ascontiguousarray(
            tokens[e].reshape(NCHUNK, 512, DO, P).transpose(0, 3, 2, 1)
        )  # [c, p, do, t']
        w1r = np.ascontiguousarray(
            w1[e].reshape(DO, P, FT, P).transpose(2, 1, 0, 3)
        )  # [ft, p, do, j]
        b1r = np.ascontiguousarray(b1[e].reshape(FT, P).T)  # [p, ft]
        w2r = np.ascontiguousarray(
            w2[e].reshape(2, FT2, P, DMO, P).transpose(3, 0, 2, 1, 4)
        )  # [dmo, half, p, fo, j]
        b2r = np.ascontiguousarray(b2[e].reshape(DMO, P).T)  # [p, dmo]
        in_maps.append({"xT": xT, "w1r": w1r, "b1r": b1r, "w2r": w2r, "b2r": b2r})
    return in_maps


def gather_out(results):
    out = np.empty((E, T, D), dtype=np.float32)
    for e in range(E):
        yT = results[e]["yT"]  # [dmo, p, t]
        out[e] = yT.transpose(2, 0, 1).reshape(T, D)
    return out.reshape(B, S, D)


def kernel(x, w1, b1, w2, b2):
    from concourse.bass_utils import run_bass_kernel_spmd

    nc = _get_nc()
    in_maps = make_in_maps(x, w1, b1, w2, b2)
    res = run_bass_kernel_spmd(nc, in_maps, core_ids=list(range(E)))
    return gather_out(res.results)
